# revision 7
# baseline (speedup 1.0000x reference)
"""AttentionMixer kernel for 8 Trainium2 NeuronCores (v4: transposed output,
no device-side epilogue, per-bank PSUM tiles).

Computes out[b,h,i,d] = sum_j softmax_j(attn_logits[b,h,i,j]) * v[b,h,j,d]
for B=2, H=16, S=2048, D=64 (f32), sharding the 32 (b,h) heads across the
8 cores (4 heads per core, no cross-core communication).

Device dataflow (per head):
  1. Logits are host-transposed to j-major bf16: lt[h, j, i]; group g of 512
     j-rows is one contiguous 2MB slab with 16KB per partition (j = g*512 +
     p*4 + q at partition p, q-th row), so each group load is 128 fat 16KB
     descriptors. Ramp loads are split across the SP HWDGE ring and the
     GPSIMD SWDGE so dispatch latency doesn't serialize the first chunks.
  2. ScalarE: exp on the whole group in one [128, 8192] bf16->bf16
     instruction (ACT is the bottleneck engine; one instr per 2MB group
     amortizes the per-instruction SBUF access latency). Head 0's first two
     groups and the last head's last group run at finer granularity to
     shorten the ramp and tail.
  3. TensorE: outT[d, i] += v_aug[j, d]^T @ expT[j, i] accumulated over the
     16 j-chunks, one single-bank PSUM tile per 512-wide i block (tile pool
     deps are whole-tile, so per-bank tiles keep the stop-matmuls from
     waiting on the previous block's PSUM read). v_aug is host-built
     [h, p, jc, 65] bf16 with a ones-column at d=64, so row 64 of outT is
     the softmax denominator. lhsT is 65 wide, not 128.
  4. DVE: copy outT PSUM f32 -> SBUF bf16 per i block; GPSIMD SWDGE stores
     the head's [65, 2048] bf16 slab (numerator-T + denominator row). The
     last head casts on DVE+ACT in parallel (ACT is idle after the last exp)
     and stores per block on the idle SP ring. The host widens to f32,
     divides by the denominator row and transposes back to [i, d].

exp is computed without max subtraction: logits are standard-normal so exp
never overflows, and softmax is shift-invariant.
"""

import numpy as np
import ml_dtypes
from concurrent.futures import ThreadPoolExecutor

import concourse.bass as bass
import concourse.mybir as mybir
from concourse import bacc
import concourse.tile as tile
from concourse.bass_utils import run_bass_kernel_spmd

P = 128  # SBUF partitions
FREE = 512  # PSUM bank width in f32 / matmul moving free dim
GROUP = 4  # j-chunks per DMA/exp group (2MB loads, [128, 8192] exp instrs)

BF16 = ml_dtypes.bfloat16


def build_nc(H: int, S: int, D: int) -> bass.Bass:
    """Single-core program: H heads, logits pre-transposed to [h, j, i]."""
    assert S % FREE == 0 and D < P
    JC = S // P  # j chunks (contraction), 16
    NG = JC // GROUP  # groups, 4
    IB = S // FREE  # i blocks (PSUM banks per head), 4
    DAUG = D + 1  # v columns + ones column (softmax denominator)
    dt = mybir.dt

    nc = bacc.Bacc()
    # lt[h, j, i]: host-transposed bf16 logits, natural j order.
    logits_t = nc.declare_dram_parameter(
        "attn_logits_t", [H, S, S], dt.bfloat16, isOutput=False
    )
    # v_aug[h, p, jc, daug]: v[h, j] at j = g*512 + p*4 + q, jc = g*4 + q,
    # with v_aug[..., D] = 1.0.
    v_aug = nc.declare_dram_parameter(
        "v_aug", [H, P, JC, DAUG], dt.bfloat16, isOutput=False
    )
    # out_t[h, d, i]: rows 0..D-1 = numerator^T, row D = denominator.
    out_t = nc.declare_dram_parameter("out_t", [H, DAUG, S], dt.bfloat16, isOutput=True)

    lt_g = logits_t[:].rearrange("h (g p q) i -> h g p q i", p=P, q=GROUP)
    lt_q = logits_t[:].rearrange("h (g p q) i -> h g q p i", p=P, q=GROUP)

    with (
        tile.TileContext(nc) as tc,
        tc.tile_pool(name="consts", bufs=1) as consts,
        tc.tile_pool(name="lpool", bufs=4) as lpool,
        tc.tile_pool(name="ppool", bufs=3) as ppool,
        tc.tile_pool(name="vpool", bufs=2) as vpool,
        tc.tile_pool(name="opool", bufs=2) as opool,
        tc.tile_pool(name="obank", bufs=1) as obank,
        tc.tile_pool(name="ps_o", bufs=2, space="PSUM") as ps_o,
    ):
        # Dummy exp up front so the ~1.3us ACT table load overlaps the
        # first DMA load instead of delaying the first real exp.
        warm = consts.tile([P, 1], dt.float32, tag="warm")
        nc.gpsimd.memset(warm[:], 0.0)
        nc.scalar.activation(warm[:], warm[:], mybir.ActivationFunctionType.Exp)

        for h in range(H):
            ramp = h == 0  # fine-grained DMA/exp on groups 0-1 of head 0
            tail_head = h == H - 1

            v_sb = vpool.tile([P, JC, DAUG], dt.bfloat16, tag="vload")
            if not ramp:
                nc.sync.dma_start(v_sb[:], v_aug[h])

            # One single-bank PSUM tile per i block (per-bank dependency
            # tracking); bufs=2 x 4 names = all 8 banks, double-buffered
            # across heads.
            o_ps = [
                ps_o.tile([P, FREE], dt.float32, name=f"ops{ib}", tag=f"ops{ib}")
                for ib in range(IB)
            ]
            if tail_head:
                o_sb = [
                    obank.tile([P, FREE], dt.bfloat16, name=f"ob{ib}", tag=f"ob{ib}")
                    for ib in range(IB)
                ]
            else:
                o_full = opool.tile([P, IB * FREE], dt.bfloat16, tag="osb")

            for g in range(NG):
                fine = ramp and g < 2
                tail_blk = tail_head and g == NG - 1
                lt_t = lpool.tile([P, GROUP, S], dt.bfloat16, tag="lt")
                pb = ppool.tile([P, GROUP, S], dt.bfloat16, tag="pb")

                # DMA: ramp groups load per 512KB j-chunk, alternating the
                # SP HWDGE ring and GPSIMD SWDGE so dispatch latency overlaps;
                # steady state loads the whole 2MB group (16KB descriptors).
                if fine:
                    if g == 0:
                        # First half-chunk gates the very first exp; loads
                        # alternate the SP HWDGE ring and the GPSIMD SWDGE,
                        # in exact consumption order (v load last: the
                        # first matmul only needs it after q0's exp).
                        nc.sync.dma_start(lt_t[:, 0, : S // 2], lt_q[h, 0, 0][:, : S // 2])
                        nc.gpsimd.dma_start(lt_t[:, 0, S // 2 :], lt_q[h, 0, 0][:, S // 2 :])
                        nc.sync.dma_start(lt_t[:, 1, :], lt_q[h, 0, 1])
                        nc.gpsimd.dma_start(lt_t[:, 2, :], lt_q[h, 0, 2])
                        nc.sync.dma_start(lt_t[:, 3, :], lt_q[h, 0, 3])
                        nc.gpsimd.dma_start(v_sb[:], v_aug[h])
                    else:
                        # Two half-group loads (8KB descriptors), one per
                        # ring, feeding two [128, 4096] exps.
                        nc.sync.dma_start(lt_t[:, 0:2, :], lt_g[h, g][:, 0:2, :])
                        nc.gpsimd.dma_start(lt_t[:, 2:4, :], lt_g[h, g][:, 2:4, :])
                else:
                    nc.sync.dma_start(lt_t[:], lt_g[h, g])

                # exp + PV matmuls. Emission per q keeps PE fed as soon as
                # each chunk's exp lands; steady-state exp is one big instr.
                def mm(q, ibs=range(IB)):
                    jc = g * GROUP + q
                    for ib in ibs:
                        nc.tensor.matmul(
                            o_ps[ib][0:DAUG, :],
                            lhsT=v_sb[:, jc, :],
                            rhs=pb[:, q, ib * FREE : (ib + 1) * FREE],
                            start=(jc == 0),
                            stop=(jc == JC - 1),
                        )

                if fine and g == 0:
                    nc.scalar.activation(
                        pb[:, 0, : S // 2], lt_t[:, 0, : S // 2],
                        mybir.ActivationFunctionType.Exp,
                    )
                    nc.scalar.activation(
                        pb[:, 0, S // 2 :], lt_t[:, 0, S // 2 :],
                        mybir.ActivationFunctionType.Exp,
                    )
                    mm(0)
                    for q in range(1, GROUP):
                        nc.scalar.activation(
                            pb[:, q, :], lt_t[:, q, :],
                            mybir.ActivationFunctionType.Exp,
                        )
                        mm(q)
                elif fine:
                    nc.scalar.activation(
                        pb[:, 0:2, :], lt_t[:, 0:2, :],
                        mybir.ActivationFunctionType.Exp,
                    )
                    mm(0)
                    mm(1)
                    nc.scalar.activation(
                        pb[:, 2:4, :], lt_t[:, 2:4, :],
                        mybir.ActivationFunctionType.Exp,
                    )
                    mm(2)
                    mm(3)
                elif tail_blk:
                    # Tail: per-chunk exps, last chunk split in i-halves.
                    # All stop-matmuls are emitted before any PSUM read;
                    # casts run on DVE and ACT in parallel (ACT is done),
                    # stores go per block on the idle SP ring + GPSIMD.
                    for q in range(GROUP - 1):
                        nc.scalar.activation(
                            pb[:, q, :], lt_t[:, q, :],
                            mybir.ActivationFunctionType.Exp,
                        )
                        mm(q)
                    qL = GROUP - 1
                    nc.scalar.activation(
                        pb[:, qL, : S // 2], lt_t[:, qL, : S // 2],
                        mybir.ActivationFunctionType.Exp,
                    )
                    mm(qL, ibs=range(IB // 2))
                    nc.vector.tensor_copy(out=o_sb[0][0:DAUG, :], in_=o_ps[0][0:DAUG, :])
                    nc.gpsimd.dma_start(out_t[h][:, 0:FREE], o_sb[0][0:DAUG, :])
                    nc.vector.tensor_copy(out=o_sb[1][0:DAUG, :], in_=o_ps[1][0:DAUG, :])
                    nc.gpsimd.dma_start(out_t[h][:, FREE : 2 * FREE], o_sb[1][0:DAUG, :])
                    nc.scalar.activation(
                        pb[:, qL, S // 2 :], lt_t[:, qL, S // 2 :],
                        mybir.ActivationFunctionType.Exp,
                    )
                    mm(qL, ibs=range(IB // 2, IB))
                    # ib2 on DVE, ib3 on ACT (activation Copy): parallel.
                    nc.vector.tensor_copy(out=o_sb[2][0:DAUG, :], in_=o_ps[2][0:DAUG, :])
                    nc.gpsimd.dma_start(out_t[h][:, 2 * FREE : 3 * FREE], o_sb[2][0:DAUG, :])
                    nc.scalar.activation(
                        o_sb[3][0:DAUG, :], o_ps[3][0:DAUG, :],
                        mybir.ActivationFunctionType.Copy,
                    )
                    nc.sync.dma_start(out_t[h][:, 3 * FREE :], o_sb[3][0:DAUG, :])
                else:
                    nc.scalar.activation(
                        pb[:], lt_t[:], mybir.ActivationFunctionType.Exp
                    )
                    for q in range(GROUP):
                        mm(q)

            if not tail_head:
                for ib in range(IB):
                    nc.vector.tensor_copy(
                        out=o_full[0:DAUG, ib * FREE : (ib + 1) * FREE],
                        in_=o_ps[ib][0:DAUG, :],
                    )
                # One [65, 2048] bf16 store per head on the GPSIMD SWDGE
                # (idle engine, separate descriptor queues: interferes with
                # neither the ACT stream nor the SP-ring loads).
                nc.gpsimd.dma_start(out_t[h], o_full[0:DAUG, :])

    nc.compile()
    return nc


def make_in_maps(v: np.ndarray, attn_logits: np.ndarray, n_cores: int = 8):
    B, H, S, D = v.shape
    heads = B * H
    hper = heads // n_cores
    JC = S // P
    NG = JC // GROUP
    DAUG = D + 1

    # v_aug[h, p, jc, daug] bf16 with j = g*512 + p*4 + q, jc = g*4 + q.
    vf = np.asarray(v, dtype=np.float32).reshape(heads, S, D)
    va = np.empty((heads, P, JC, DAUG), dtype=BF16)
    va[..., D] = 1.0
    va[..., :D] = (
        vf.reshape(heads, NG, P, GROUP, D)
        .transpose(0, 2, 1, 3, 4)
        .reshape(heads, P, JC, D)
    )

    # lt[h, j, i] = bf16(logits[h, i, j]); blocked transpose per head.
    lf = np.asarray(attn_logits, dtype=np.float32).reshape(heads, S, S)
    lt_all = np.empty((heads, S, S), dtype=BF16)

    def do_head(h):
        A = lf[h].astype(BF16)  # [i, j]
        Ah = lt_all[h]
        for jb in range(0, S, 256):
            Ah[jb : jb + 256] = A[:, jb : jb + 256].T

    with ThreadPoolExecutor(8) as ex:
        list(ex.map(do_head, range(heads)))

    return [
        {
            "v_aug": va[c * hper : (c + 1) * hper],
            "attn_logits_t": lt_all[c * hper : (c + 1) * hper],
        }
        for c in range(n_cores)
    ]


def unshard_output(results, B, H, S, D):
    """results: per-core dicts with out_t [hper, D+1, S] bf16."""
    n_cores = len(results)
    out_t = np.concatenate(
        [np.asarray(results[c]["out_t"]) for c in range(n_cores)], axis=0
    ).astype(np.float32)  # [heads, D+1, S]
    num = out_t[:, :D, :]  # [h, d, i]
    den = out_t[:, D, :]  # [h, i]
    out = (num / den[:, None, :]).transpose(0, 2, 1)  # [h, i, d]
    return np.ascontiguousarray(out).reshape(B, H, S, D).astype(np.float32)


_NC_CACHE: dict = {}


def _get_nc(H: int, S: int, D: int) -> bass.Bass:
    key = (H, S, D)
    if key not in _NC_CACHE:
        _NC_CACHE[key] = build_nc(H, S, D)
    return _NC_CACHE[key]


def kernel(v: np.ndarray, attn_logits: np.ndarray) -> np.ndarray:
    B, H, S, D = v.shape
    assert attn_logits.shape == (B, H, S, S)
    n_cores = 8
    heads = B * H
    assert heads % n_cores == 0
    hper = heads // n_cores

    nc = _get_nc(hper, S, D)
    in_maps = make_in_maps(v, attn_logits, n_cores)
    res = run_bass_kernel_spmd(nc, in_maps, core_ids=list(range(n_cores)))
    return unshard_output(res.results, B, H, S, D)


# revision 8
# speedup vs baseline: 1.0004x; 1.0004x over previous
"""AttentionMixer kernel for 8 Trainium2 NeuronCores (v4: transposed output,
no device-side epilogue, per-bank PSUM tiles).

Computes out[b,h,i,d] = sum_j softmax_j(attn_logits[b,h,i,j]) * v[b,h,j,d]
for B=2, H=16, S=2048, D=64 (f32), sharding the 32 (b,h) heads across the
8 cores (4 heads per core, no cross-core communication).

Device dataflow (per head):
  1. Logits are host-transposed to j-major bf16: lt[h, j, i]; group g of 512
     j-rows is one contiguous 2MB slab with 16KB per partition (j = g*512 +
     p*4 + q at partition p, q-th row), so each group load is 128 fat 16KB
     descriptors. Ramp loads are split across the SP HWDGE ring and the
     GPSIMD SWDGE so dispatch latency doesn't serialize the first chunks.
  2. ScalarE: exp on the whole group in one [128, 8192] bf16->bf16
     instruction (ACT is the bottleneck engine; one instr per 2MB group
     amortizes the per-instruction SBUF access latency). Head 0's first two
     groups and the last head's last group run at finer granularity to
     shorten the ramp and tail.
  3. TensorE: outT[d, i] += v_aug[j, d]^T @ expT[j, i] accumulated over the
     16 j-chunks, one single-bank PSUM tile per 512-wide i block (tile pool
     deps are whole-tile, so per-bank tiles keep the stop-matmuls from
     waiting on the previous block's PSUM read). v_aug is host-built
     [h, p, jc, 65] bf16 with a ones-column at d=64, so row 64 of outT is
     the softmax denominator. lhsT is 65 wide, not 128.
  4. DVE: copy outT PSUM f32 -> SBUF bf16 per i block; GPSIMD SWDGE stores
     the head's [65, 2048] bf16 slab (numerator-T + denominator row). The
     last head casts on DVE+ACT in parallel (ACT is idle after the last exp)
     and stores per block on the idle SP ring. The host widens to f32,
     divides by the denominator row and transposes back to [i, d].

exp is computed without max subtraction: logits are standard-normal so exp
never overflows, and softmax is shift-invariant.
"""

import numpy as np
import ml_dtypes
from concurrent.futures import ThreadPoolExecutor

import concourse.bass as bass
import concourse.mybir as mybir
from concourse import bacc
import concourse.tile as tile
from concourse.bass_utils import run_bass_kernel_spmd

P = 128  # SBUF partitions
FREE = 512  # PSUM bank width in f32 / matmul moving free dim
GROUP = 4  # j-chunks per DMA/exp group (2MB loads, [128, 8192] exp instrs)

BF16 = ml_dtypes.bfloat16


def build_nc(H: int, S: int, D: int) -> bass.Bass:
    """Single-core program: H heads, logits pre-transposed to [h, j, i]."""
    assert S % FREE == 0 and D < P
    JC = S // P  # j chunks (contraction), 16
    NG = JC // GROUP  # groups, 4
    IB = S // FREE  # i blocks (PSUM banks per head), 4
    DAUG = D + 1  # v columns + ones column (softmax denominator)
    dt = mybir.dt

    nc = bacc.Bacc()
    # lt[h, j, i]: host-transposed bf16 logits, natural j order.
    logits_t = nc.declare_dram_parameter(
        "attn_logits_t", [H, S, S], dt.bfloat16, isOutput=False
    )
    # v_aug[h, p, jc, daug]: v[h, j] at j = g*512 + p*4 + q, jc = g*4 + q,
    # with v_aug[..., D] = 1.0.
    v_aug = nc.declare_dram_parameter(
        "v_aug", [H, P, JC, DAUG], dt.bfloat16, isOutput=False
    )
    # out_t[h, d, i]: rows 0..D-1 = numerator^T, row D = denominator.
    out_t = nc.declare_dram_parameter("out_t", [H, DAUG, S], dt.bfloat16, isOutput=True)

    lt_g = logits_t[:].rearrange("h (g p q) i -> h g p q i", p=P, q=GROUP)
    lt_q = logits_t[:].rearrange("h (g p q) i -> h g q p i", p=P, q=GROUP)

    with (
        tile.TileContext(nc) as tc,
        tc.tile_pool(name="consts", bufs=1) as consts,
        tc.tile_pool(name="lpool", bufs=4) as lpool,
        tc.tile_pool(name="ppool", bufs=3) as ppool,
        tc.tile_pool(name="vpool", bufs=2) as vpool,
        tc.tile_pool(name="opool", bufs=2) as opool,
        tc.tile_pool(name="obank", bufs=1) as obank,
        tc.tile_pool(name="ps_o", bufs=2, space="PSUM") as ps_o,
    ):
        # Dummy exp up front so the ~1.3us ACT table load overlaps the
        # first DMA load instead of delaying the first real exp.
        warm = consts.tile([P, 1], dt.float32, tag="warm")
        nc.gpsimd.memset(warm[:], 0.0)
        nc.scalar.activation(warm[:], warm[:], mybir.ActivationFunctionType.Exp)

        for h in range(H):
            ramp = h == 0  # fine-grained DMA/exp on groups 0-1 of head 0
            tail_head = h == H - 1

            v_sb = vpool.tile([P, JC, DAUG], dt.bfloat16, tag="vload")
            if not ramp:
                nc.sync.dma_start(v_sb[:], v_aug[h])

            # One single-bank PSUM tile per i block (per-bank dependency
            # tracking); bufs=2 x 4 names = all 8 banks, double-buffered
            # across heads.
            o_ps = [
                ps_o.tile([P, FREE], dt.float32, name=f"ops{ib}", tag=f"ops{ib}")
                for ib in range(IB)
            ]
            if tail_head:
                o_sb = [
                    obank.tile([P, FREE], dt.bfloat16, name=f"ob{ib}", tag=f"ob{ib}")
                    for ib in range(IB)
                ]
            else:
                o_full = opool.tile([P, IB * FREE], dt.bfloat16, tag="osb")

            for g in range(NG):
                fine = ramp and g < 2
                tail_blk = tail_head and g == NG - 1
                lt_t = lpool.tile([P, GROUP, S], dt.bfloat16, tag="lt")
                pb = ppool.tile([P, GROUP, S], dt.bfloat16, tag="pb")

                # DMA: ramp groups load per 512KB j-chunk, alternating the
                # SP HWDGE ring and GPSIMD SWDGE so dispatch latency overlaps;
                # steady state loads the whole 2MB group (16KB descriptors).
                if fine:
                    if g == 0:
                        # First half-chunk gates the very first exp; loads
                        # alternate the SP HWDGE ring and the GPSIMD SWDGE,
                        # in exact consumption order (v load last: the
                        # first matmul only needs it after q0's exp).
                        nc.sync.dma_start(lt_t[:, 0, : S // 2], lt_q[h, 0, 0][:, : S // 2])
                        nc.gpsimd.dma_start(lt_t[:, 0, S // 2 :], lt_q[h, 0, 0][:, S // 2 :])
                        nc.sync.dma_start(lt_t[:, 1, :], lt_q[h, 0, 1])
                        nc.gpsimd.dma_start(lt_t[:, 2, :], lt_q[h, 0, 2])
                        nc.sync.dma_start(lt_t[:, 3, :], lt_q[h, 0, 3])
                        nc.gpsimd.dma_start(v_sb[:], v_aug[h])
                    else:
                        for q in range(GROUP):
                            eng = nc.sync if q % 2 == 0 else nc.gpsimd
                            eng.dma_start(lt_t[:, q, :], lt_q[h, g, q])
                else:
                    nc.sync.dma_start(lt_t[:], lt_g[h, g])

                # exp + PV matmuls. Emission per q keeps PE fed as soon as
                # each chunk's exp lands; steady-state exp is one big instr.
                def mm(q, ibs=range(IB)):
                    jc = g * GROUP + q
                    for ib in ibs:
                        nc.tensor.matmul(
                            o_ps[ib][0:DAUG, :],
                            lhsT=v_sb[:, jc, :],
                            rhs=pb[:, q, ib * FREE : (ib + 1) * FREE],
                            start=(jc == 0),
                            stop=(jc == JC - 1),
                        )

                if fine and g == 0:
                    nc.scalar.activation(
                        pb[:, 0, : S // 2], lt_t[:, 0, : S // 2],
                        mybir.ActivationFunctionType.Exp,
                    )
                    nc.scalar.activation(
                        pb[:, 0, S // 2 :], lt_t[:, 0, S // 2 :],
                        mybir.ActivationFunctionType.Exp,
                    )
                    mm(0)
                    for q in range(1, GROUP):
                        nc.scalar.activation(
                            pb[:, q, :], lt_t[:, q, :],
                            mybir.ActivationFunctionType.Exp,
                        )
                        mm(q)
                elif fine:
                    for q in range(GROUP):
                        nc.scalar.activation(
                            pb[:, q, :], lt_t[:, q, :],
                            mybir.ActivationFunctionType.Exp,
                        )
                        mm(q)
                elif tail_blk:
                    # Tail: per-chunk exps, last chunk split in i-halves.
                    # All stop-matmuls are emitted before any PSUM read;
                    # casts run on DVE and ACT in parallel (ACT is done),
                    # stores go per block on the idle SP ring + GPSIMD.
                    for q in range(GROUP - 1):
                        nc.scalar.activation(
                            pb[:, q, :], lt_t[:, q, :],
                            mybir.ActivationFunctionType.Exp,
                        )
                        mm(q)
                    qL = GROUP - 1
                    nc.scalar.activation(
                        pb[:, qL, : S // 2], lt_t[:, qL, : S // 2],
                        mybir.ActivationFunctionType.Exp,
                    )
                    mm(qL, ibs=range(IB // 2))
                    nc.vector.tensor_copy(out=o_sb[0][0:DAUG, :], in_=o_ps[0][0:DAUG, :])
                    nc.gpsimd.dma_start(out_t[h][:, 0:FREE], o_sb[0][0:DAUG, :])
                    nc.vector.tensor_copy(out=o_sb[1][0:DAUG, :], in_=o_ps[1][0:DAUG, :])
                    nc.gpsimd.dma_start(out_t[h][:, FREE : 2 * FREE], o_sb[1][0:DAUG, :])
                    nc.scalar.activation(
                        pb[:, qL, S // 2 :], lt_t[:, qL, S // 2 :],
                        mybir.ActivationFunctionType.Exp,
                    )
                    mm(qL, ibs=range(IB // 2, IB))
                    # ib2 on DVE, ib3 on ACT (activation Copy): parallel.
                    nc.vector.tensor_copy(out=o_sb[2][0:DAUG, :], in_=o_ps[2][0:DAUG, :])
                    nc.gpsimd.dma_start(out_t[h][:, 2 * FREE : 3 * FREE], o_sb[2][0:DAUG, :])
                    nc.scalar.activation(
                        o_sb[3][0:DAUG, :], o_ps[3][0:DAUG, :],
                        mybir.ActivationFunctionType.Copy,
                    )
                    nc.sync.dma_start(out_t[h][:, 3 * FREE :], o_sb[3][0:DAUG, :])
                else:
                    nc.scalar.activation(
                        pb[:], lt_t[:], mybir.ActivationFunctionType.Exp
                    )
                    for q in range(GROUP):
                        mm(q)

            if not tail_head:
                for ib in range(IB):
                    nc.vector.tensor_copy(
                        out=o_full[0:DAUG, ib * FREE : (ib + 1) * FREE],
                        in_=o_ps[ib][0:DAUG, :],
                    )
                # One [65, 2048] bf16 store per head on the GPSIMD SWDGE
                # (idle engine, separate descriptor queues: interferes with
                # neither the ACT stream nor the SP-ring loads).
                nc.gpsimd.dma_start(out_t[h], o_full[0:DAUG, :])

    nc.compile()
    return nc


def make_in_maps(v: np.ndarray, attn_logits: np.ndarray, n_cores: int = 8):
    B, H, S, D = v.shape
    heads = B * H
    hper = heads // n_cores
    JC = S // P
    NG = JC // GROUP
    DAUG = D + 1

    # v_aug[h, p, jc, daug] bf16 with j = g*512 + p*4 + q, jc = g*4 + q.
    vf = np.asarray(v, dtype=np.float32).reshape(heads, S, D)
    va = np.empty((heads, P, JC, DAUG), dtype=BF16)
    va[..., D] = 1.0
    va[..., :D] = (
        vf.reshape(heads, NG, P, GROUP, D)
        .transpose(0, 2, 1, 3, 4)
        .reshape(heads, P, JC, D)
    )

    # lt[h, j, i] = bf16(logits[h, i, j]); blocked transpose per head.
    lf = np.asarray(attn_logits, dtype=np.float32).reshape(heads, S, S)
    lt_all = np.empty((heads, S, S), dtype=BF16)

    def do_head(h):
        A = lf[h].astype(BF16)  # [i, j]
        Ah = lt_all[h]
        for jb in range(0, S, 256):
            Ah[jb : jb + 256] = A[:, jb : jb + 256].T

    with ThreadPoolExecutor(8) as ex:
        list(ex.map(do_head, range(heads)))

    return [
        {
            "v_aug": va[c * hper : (c + 1) * hper],
            "attn_logits_t": lt_all[c * hper : (c + 1) * hper],
        }
        for c in range(n_cores)
    ]


def unshard_output(results, B, H, S, D):
    """results: per-core dicts with out_t [hper, D+1, S] bf16."""
    n_cores = len(results)
    out_t = np.concatenate(
        [np.asarray(results[c]["out_t"]) for c in range(n_cores)], axis=0
    ).astype(np.float32)  # [heads, D+1, S]
    num = out_t[:, :D, :]  # [h, d, i]
    den = out_t[:, D, :]  # [h, i]
    out = (num / den[:, None, :]).transpose(0, 2, 1)  # [h, i, d]
    return np.ascontiguousarray(out).reshape(B, H, S, D).astype(np.float32)


_NC_CACHE: dict = {}


def _get_nc(H: int, S: int, D: int) -> bass.Bass:
    key = (H, S, D)
    if key not in _NC_CACHE:
        _NC_CACHE[key] = build_nc(H, S, D)
    return _NC_CACHE[key]


def kernel(v: np.ndarray, attn_logits: np.ndarray) -> np.ndarray:
    B, H, S, D = v.shape
    assert attn_logits.shape == (B, H, S, S)
    n_cores = 8
    heads = B * H
    assert heads % n_cores == 0
    hper = heads // n_cores

    nc = _get_nc(hper, S, D)
    in_maps = make_in_maps(v, attn_logits, n_cores)
    res = run_bass_kernel_spmd(nc, in_maps, core_ids=list(range(n_cores)))
    return unshard_output(res.results, B, H, S, D)


# revision 9
# speedup vs baseline: 1.0314x; 1.0309x over previous
"""AttentionMixer kernel for 8 Trainium2 NeuronCores (v4: transposed output,
no device-side epilogue, per-bank PSUM tiles).

Computes out[b,h,i,d] = sum_j softmax_j(attn_logits[b,h,i,j]) * v[b,h,j,d]
for B=2, H=16, S=2048, D=64 (f32), sharding the 32 (b,h) heads across the
8 cores (4 heads per core, no cross-core communication).

Device dataflow (per head):
  1. Logits are host-transposed to j-major bf16: lt[h, j, i]; group g of 512
     j-rows is one contiguous 2MB slab with 16KB per partition (j = g*512 +
     p*4 + q at partition p, q-th row), so each group load is 128 fat 16KB
     descriptors. Ramp loads are split across the SP HWDGE ring and the
     GPSIMD SWDGE so dispatch latency doesn't serialize the first chunks.
  2. ScalarE: exp on the whole group in one [128, 8192] bf16->bf16
     instruction (ACT is the bottleneck engine; one instr per 2MB group
     amortizes the per-instruction SBUF access latency). Head 0's first two
     groups and the last head's last group run at finer granularity to
     shorten the ramp and tail.
  3. TensorE: outT[d, i] += v_aug[j, d]^T @ expT[j, i] accumulated over the
     16 j-chunks, one single-bank PSUM tile per 512-wide i block (tile pool
     deps are whole-tile, so per-bank tiles keep the stop-matmuls from
     waiting on the previous block's PSUM read). v_aug is host-built
     [h, p, jc, 65] bf16 with a ones-column at d=64, so row 64 of outT is
     the softmax denominator. lhsT is 65 wide, not 128.
  4. DVE: copy outT PSUM f32 -> SBUF bf16 per i block; GPSIMD SWDGE stores
     the head's [65, 2048] bf16 slab (numerator-T + denominator row). The
     last head casts on DVE+ACT in parallel (ACT is idle after the last exp)
     and stores per block on the idle SP ring. The host widens to f32,
     divides by the denominator row and transposes back to [i, d].

exp is computed without max subtraction: logits are standard-normal so exp
never overflows, and softmax is shift-invariant.
"""

import numpy as np
import ml_dtypes
from concurrent.futures import ThreadPoolExecutor

import concourse.bass as bass
import concourse.mybir as mybir
from concourse import bacc
import concourse.tile as tile
from concourse.bass_utils import run_bass_kernel_spmd

P = 128  # SBUF partitions
FREE = 512  # PSUM bank width in f32 / matmul moving free dim
GROUP = 4  # j-chunks per DMA/exp group (2MB loads, [128, 8192] exp instrs)

BF16 = ml_dtypes.bfloat16


def build_nc(H: int, S: int, D: int) -> bass.Bass:
    """Single-core program: H heads, logits pre-transposed to [h, j, i]."""
    assert S % FREE == 0 and D < P
    JC = S // P  # j chunks (contraction), 16
    NG = JC // GROUP  # groups, 4
    IB = S // FREE  # i blocks (PSUM banks per head), 4
    DAUG = D + 1  # v columns + ones column (softmax denominator)
    dt = mybir.dt

    nc = bacc.Bacc()
    # lt[h, j, i]: host-transposed bf16 logits, natural j order.
    logits_t = nc.declare_dram_parameter(
        "attn_logits_t", [H, S, S], dt.bfloat16, isOutput=False
    )
    # v_aug[h, p, jc, daug]: v[h, j] at j = g*512 + p*4 + q, jc = g*4 + q,
    # with v_aug[..., D] = 1.0.
    v_aug = nc.declare_dram_parameter(
        "v_aug", [H, P, JC, DAUG], dt.bfloat16, isOutput=False
    )
    # out_t[h, d, i]: rows 0..D-1 = numerator^T, row D = denominator.
    out_t = nc.declare_dram_parameter("out_t", [H, DAUG, S], dt.bfloat16, isOutput=True)

    lt_g = logits_t[:].rearrange("h (g p q) i -> h g p q i", p=P, q=GROUP)
    lt_q = logits_t[:].rearrange("h (g p q) i -> h g q p i", p=P, q=GROUP)

    with (
        tile.TileContext(nc) as tc,
        tc.tile_pool(name="consts", bufs=1) as consts,
        tc.tile_pool(name="lpool", bufs=4) as lpool,
        tc.tile_pool(name="ppool", bufs=3) as ppool,
        tc.tile_pool(name="vpool", bufs=2) as vpool,
        tc.tile_pool(name="opool", bufs=2) as opool,
        tc.tile_pool(name="obank", bufs=1) as obank,
        tc.tile_pool(name="ps_o", bufs=2, space="PSUM") as ps_o,
    ):
        # Dummy exp up front so the ~1.3us ACT table load overlaps the
        # first DMA load instead of delaying the first real exp.
        warm = consts.tile([P, 1], dt.float32, tag="warm")
        nc.gpsimd.memset(warm[:], 0.0)
        nc.scalar.activation(warm[:], warm[:], mybir.ActivationFunctionType.Exp)

        for h in range(H):
            ramp = h == 0  # fine-grained DMA/exp on groups 0-1 of head 0
            tail_head = h == H - 1

            v_sb = vpool.tile([P, JC, DAUG], dt.bfloat16, tag="vload")
            if not ramp:
                nc.sync.dma_start(v_sb[:], v_aug[h])

            # One single-bank PSUM tile per i block (per-bank dependency
            # tracking); bufs=2 x 4 names = all 8 banks, double-buffered
            # across heads.
            o_ps = [
                ps_o.tile([P, FREE], dt.float32, name=f"ops{ib}", tag=f"ops{ib}")
                for ib in range(IB)
            ]
            if tail_head:
                o_sb = [
                    obank.tile([P, FREE], dt.bfloat16, name=f"ob{ib}", tag=f"ob{ib}")
                    for ib in range(IB)
                ]
            else:
                o_full = opool.tile([P, IB * FREE], dt.bfloat16, tag="osb")

            for g in range(NG):
                fine = ramp and g < 2
                tail_blk = tail_head and g == NG - 1
                lt_t = lpool.tile([P, GROUP, S], dt.bfloat16, tag="lt")
                pb = ppool.tile([P, GROUP, S], dt.bfloat16, tag="pb")

                # DMA: ramp groups load per 512KB j-chunk, alternating the
                # SP HWDGE ring and GPSIMD SWDGE so dispatch latency overlaps;
                # steady state loads the whole 2MB group (16KB descriptors).
                if fine:
                    if g == 0:
                        # First half-chunk gates the very first exp; loads
                        # alternate the SP HWDGE ring and the GPSIMD SWDGE,
                        # in exact consumption order (v load last: the
                        # first matmul only needs it after q0's exp).
                        nc.sync.dma_start(lt_t[:, 0, : S // 2], lt_q[h, 0, 0][:, : S // 2])
                        nc.gpsimd.dma_start(lt_t[:, 0, S // 2 :], lt_q[h, 0, 0][:, S // 2 :])
                        nc.sync.dma_start(lt_t[:, 1, :], lt_q[h, 0, 1])
                        nc.gpsimd.dma_start(v_sb[:], v_aug[h])
                        nc.sync.dma_start(lt_t[:, 2, :], lt_q[h, 0, 2])
                        nc.gpsimd.dma_start(lt_t[:, 3, :], lt_q[h, 0, 3])
                    else:
                        for q in range(GROUP):
                            eng = nc.sync if q % 2 == 0 else nc.gpsimd
                            eng.dma_start(lt_t[:, q, :], lt_q[h, g, q])
                else:
                    nc.sync.dma_start(lt_t[:], lt_g[h, g])

                # exp + PV matmuls. Emission per q keeps PE fed as soon as
                # each chunk's exp lands; steady-state exp is one big instr.
                def mm(q, ibs=range(IB)):
                    jc = g * GROUP + q
                    for ib in ibs:
                        nc.tensor.matmul(
                            o_ps[ib][0:DAUG, :],
                            lhsT=v_sb[:, jc, :],
                            rhs=pb[:, q, ib * FREE : (ib + 1) * FREE],
                            start=(jc == 0),
                            stop=(jc == JC - 1),
                        )

                if fine and g == 0:
                    nc.scalar.activation(
                        pb[:, 0, : S // 2], lt_t[:, 0, : S // 2],
                        mybir.ActivationFunctionType.Exp,
                    )
                    nc.scalar.activation(
                        pb[:, 0, S // 2 :], lt_t[:, 0, S // 2 :],
                        mybir.ActivationFunctionType.Exp,
                    )
                    mm(0)
                    for q in range(1, GROUP):
                        nc.scalar.activation(
                            pb[:, q, :], lt_t[:, q, :],
                            mybir.ActivationFunctionType.Exp,
                        )
                        mm(q)
                elif fine:
                    for q in range(GROUP):
                        nc.scalar.activation(
                            pb[:, q, :], lt_t[:, q, :],
                            mybir.ActivationFunctionType.Exp,
                        )
                        mm(q)
                elif tail_blk:
                    # Tail: per-chunk exps, last chunk split in i-halves.
                    # All stop-matmuls are emitted before any PSUM read;
                    # casts run on DVE and ACT in parallel (ACT is done),
                    # stores go per block on the idle SP ring + GPSIMD.
                    for q in range(GROUP - 1):
                        nc.scalar.activation(
                            pb[:, q, :], lt_t[:, q, :],
                            mybir.ActivationFunctionType.Exp,
                        )
                        mm(q)
                    qL = GROUP - 1
                    nc.scalar.activation(
                        pb[:, qL, : S // 2], lt_t[:, qL, : S // 2],
                        mybir.ActivationFunctionType.Exp,
                    )
                    mm(qL, ibs=range(IB // 2))
                    nc.vector.tensor_copy(out=o_sb[0][0:DAUG, :], in_=o_ps[0][0:DAUG, :])
                    nc.gpsimd.dma_start(out_t[h][:, 0:FREE], o_sb[0][0:DAUG, :])
                    nc.vector.tensor_copy(out=o_sb[1][0:DAUG, :], in_=o_ps[1][0:DAUG, :])
                    nc.gpsimd.dma_start(out_t[h][:, FREE : 2 * FREE], o_sb[1][0:DAUG, :])
                    nc.scalar.activation(
                        pb[:, qL, S // 2 :], lt_t[:, qL, S // 2 :],
                        mybir.ActivationFunctionType.Exp,
                    )
                    mm(qL, ibs=range(IB // 2, IB))
                    # ib2 on DVE, ib3 on ACT (activation Copy): parallel.
                    nc.vector.tensor_copy(out=o_sb[2][0:DAUG, :], in_=o_ps[2][0:DAUG, :])
                    nc.gpsimd.dma_start(out_t[h][:, 2 * FREE : 3 * FREE], o_sb[2][0:DAUG, :])
                    nc.scalar.activation(
                        o_sb[3][0:DAUG, :], o_ps[3][0:DAUG, :],
                        mybir.ActivationFunctionType.Copy,
                    )
                    nc.sync.dma_start(out_t[h][:, 3 * FREE :], o_sb[3][0:DAUG, :])
                else:
                    nc.scalar.activation(
                        pb[:], lt_t[:], mybir.ActivationFunctionType.Exp
                    )
                    for q in range(GROUP):
                        mm(q)

            if not tail_head:
                for ib in range(IB):
                    nc.vector.tensor_copy(
                        out=o_full[0:DAUG, ib * FREE : (ib + 1) * FREE],
                        in_=o_ps[ib][0:DAUG, :],
                    )
                # One [65, 2048] bf16 store per head on the GPSIMD SWDGE
                # (idle engine, separate descriptor queues: interferes with
                # neither the ACT stream nor the SP-ring loads).
                nc.gpsimd.dma_start(out_t[h], o_full[0:DAUG, :])

    nc.compile()
    return nc


def make_in_maps(v: np.ndarray, attn_logits: np.ndarray, n_cores: int = 8):
    B, H, S, D = v.shape
    heads = B * H
    hper = heads // n_cores
    JC = S // P
    NG = JC // GROUP
    DAUG = D + 1

    # v_aug[h, p, jc, daug] bf16 with j = g*512 + p*4 + q, jc = g*4 + q.
    vf = np.asarray(v, dtype=np.float32).reshape(heads, S, D)
    va = np.empty((heads, P, JC, DAUG), dtype=BF16)
    va[..., D] = 1.0
    va[..., :D] = (
        vf.reshape(heads, NG, P, GROUP, D)
        .transpose(0, 2, 1, 3, 4)
        .reshape(heads, P, JC, D)
    )

    # lt[h, j, i] = bf16(logits[h, i, j]); blocked transpose per head.
    lf = np.asarray(attn_logits, dtype=np.float32).reshape(heads, S, S)
    lt_all = np.empty((heads, S, S), dtype=BF16)

    def do_head(h):
        A = lf[h].astype(BF16)  # [i, j]
        Ah = lt_all[h]
        for jb in range(0, S, 256):
            Ah[jb : jb + 256] = A[:, jb : jb + 256].T

    with ThreadPoolExecutor(8) as ex:
        list(ex.map(do_head, range(heads)))

    return [
        {
            "v_aug": va[c * hper : (c + 1) * hper],
            "attn_logits_t": lt_all[c * hper : (c + 1) * hper],
        }
        for c in range(n_cores)
    ]


def unshard_output(results, B, H, S, D):
    """results: per-core dicts with out_t [hper, D+1, S] bf16."""
    n_cores = len(results)
    out_t = np.concatenate(
        [np.asarray(results[c]["out_t"]) for c in range(n_cores)], axis=0
    ).astype(np.float32)  # [heads, D+1, S]
    num = out_t[:, :D, :]  # [h, d, i]
    den = out_t[:, D, :]  # [h, i]
    out = (num / den[:, None, :]).transpose(0, 2, 1)  # [h, i, d]
    return np.ascontiguousarray(out).reshape(B, H, S, D).astype(np.float32)


_NC_CACHE: dict = {}


def _get_nc(H: int, S: int, D: int) -> bass.Bass:
    key = (H, S, D)
    if key not in _NC_CACHE:
        _NC_CACHE[key] = build_nc(H, S, D)
    return _NC_CACHE[key]


def kernel(v: np.ndarray, attn_logits: np.ndarray) -> np.ndarray:
    B, H, S, D = v.shape
    assert attn_logits.shape == (B, H, S, S)
    n_cores = 8
    heads = B * H
    assert heads % n_cores == 0
    hper = heads // n_cores

    nc = _get_nc(hper, S, D)
    in_maps = make_in_maps(v, attn_logits, n_cores)
    res = run_bass_kernel_spmd(nc, in_maps, core_ids=list(range(n_cores)))
    return unshard_output(res.results, B, H, S, D)
